# revision 3
# baseline (speedup 1.0000x reference)
"""DiffAttn kernel for 8 Trainium2 NeuronCores.

Problem: out = softmax(Q1 K1^T / sqrt(d)) V - lam * softmax(Q2 K2^T / sqrt(d)) V
with Q = X W_q, K = X W_k, V = X W_v;  X [2, 4096, 1024], W [1024, 128], d = 64.

Sharding: 8 cores = (batch b, query-chunk qc) with b = core // 4, qc = core % 4.
Each core receives its batch's X rolled so that its 1024 query rows come first
(attention is permutation-invariant over keys, so K/V can be computed over the
rolled sequence).  Each core computes the full K/V projection for its batch,
Q for its query chunk, and two-branch flash attention without max-subtraction
(scores are ~N(0,1); exp is safe in fp32), normalizing at the end:
out = U1 / r1 - lam * U2 / r2 with U_i = exp(S_i) V and r_i the row sums.

Matmuls run in float32r (verified ~1.6e-4 relative matmul error on HW).
"""

import sys

if '/opt/trn_rl_repo' not in sys.path:
    sys.path.insert(0, '/opt/trn_rl_repo')

import numpy as np

B, S, DIN, D = 2, 4096, 1024, 64
TD = 2 * D            # 128: both branches' head dims, packed on partitions
NQ = S // 4           # 1024 query rows per core
ST = 512              # seq tile (projection granularity)
NST = S // ST         # 8
QT = 512              # query tile in attention
NQT = NQ // QT        # 2
KT = 128              # key tile in attention
NKT = S // KT         # 32
NDC = DIN // 128      # 8 contraction chunks


def build_nc():
    import concourse.bacc as bacc
    import concourse.mybir as mybir
    from concourse.tile import TileContext
    from concourse.masks import make_identity

    F32 = mybir.dt.float32
    F32R = mybir.dt.float32r
    AF = mybir.ActivationFunctionType

    nc = bacc.Bacc("TRN2", target_bir_lowering=False)
    X_t = nc.dram_tensor("X", [S, DIN], F32, kind="ExternalInput")
    Wq_t = nc.dram_tensor("Wq", [DIN, TD], F32, kind="ExternalInput")
    Wk_t = nc.dram_tensor("Wk", [DIN, TD], F32, kind="ExternalInput")
    Wv_t = nc.dram_tensor("Wv", [DIN, TD], F32, kind="ExternalInput")
    lam_t = nc.dram_tensor("lam", [1, 1], F32, kind="ExternalInput")
    out_t = nc.dram_tensor("out", [NQ, TD], F32, kind="ExternalOutput")

    with TileContext(nc) as tc:
        with tc.tile_pool(name="consts", bufs=1) as consts, \
             tc.tile_pool(name="kv", bufs=1) as kv:
            # weights, [128 part = contraction sub-chunk, NDC chunks, TD]
            w_sb = {}
            for name, t in (("wq", Wq_t), ("wk", Wk_t), ("wv", Wv_t)):
                w = consts.tile([128, NDC, TD], F32R, tag=name)
                nc.sync.dma_start(
                    out=w,
                    in_=t.ap().rearrange("(c p) n -> p c n", p=128).bitcast(F32R))
                w_sb[name] = w
            ident = consts.tile([128, 128], F32, tag="ident")
            make_identity(nc, ident)
            ident1 = consts.tile([1, 1], F32, tag="ident1")
            nc.vector.memset(ident1, 1.0)
            ones32 = consts.tile([128, 1], F32, tag="ones32")
            nc.vector.memset(ones32, 1.0)
            ones = consts.tile([128, 1], F32R, tag="ones")
            nc.scalar.copy(out=ones, in_=ones32)
            lam_sb = consts.tile([128, 1], F32, tag="lam")
            nc.gpsimd.dma_start(
                out=lam_sb, in_=lam_t.ap().partition_broadcast(128))

            # per-batch projections, kept resident through the attention phase
            kT = kv.tile([128, S], F32R, tag="kT")       # K^T [2d, S]
            qT = kv.tile([128, NQ], F32R, tag="qT")      # Q^T [2d, NQ]
            vT = kv.tile([128, S], F32, tag="vT")        # V^T [2d, S] (staging)
            v_sb = kv.tile([128, NKT, TD], F32R, tag="v")  # V natural, k-tiled

            # ---------------- Phase A: projections ----------------
            with tc.tile_pool(name="xn", bufs=6) as xn_pool, \
                 tc.tile_pool(name="xt", bufs=2) as xt_pool, \
                 tc.tile_pool(name="pps", bufs=1, space="PSUM") as pps, \
                 tc.tile_pool(name="tps", bufs=4, space="PSUM") as tps:
                for st in range(NST):
                    # natural X rows for this seq tile (4 x 128 rows)
                    xns = []
                    for sp in range(4):
                        xn = xn_pool.tile([128, DIN], F32, tag="xn")
                        r0 = st * ST + sp * 128
                        nc.sync.dma_start(out=xn, in_=X_t.ap()[r0:r0 + 128, :])
                        xns.append(xn)
                    # transpose to X^T tile [128 (din chunk), NDC, ST]
                    xt = xt_pool.tile([128, NDC, ST], F32R, tag="xt")
                    for dc in range(NDC):
                        tp = tps.tile([128, ST], F32, tag="tp")
                        for sp in range(4):
                            nc.tensor.matmul(
                                tp[:, sp * 128:(sp + 1) * 128],
                                xns[sp][:, dc * 128:(dc + 1) * 128],
                                ident, is_transpose=True,
                                start=(sp == 0), stop=(sp == 3))
                        nc.vector.tensor_copy(xt[:, dc, :], tp)

                    # K^T and V^T (and Q^T for the first NQ rows)
                    for wname, dst in (("wk", kT), ("wv", vT), ("wq", qT)):
                        if wname == "wq" and st * ST >= NQ:
                            continue
                        ps = pps.tile([128, ST], F32, tag="p" + wname)
                        for dc in range(NDC):
                            nc.tensor.matmul(
                                ps, w_sb[wname][:, dc, :], xt[:, dc, :],
                                start=(dc == 0), stop=(dc == NDC - 1))
                        nc.scalar.copy(
                            out=dst[:, st * ST:(st + 1) * ST], in_=ps)

                    # V natural blocks for this seq tile
                    for j in range(4):
                        vp = tps.tile([128, TD], F32, tag="tp")
                        c0 = st * ST + j * 128
                        nc.tensor.transpose(vp, vT[:, c0:c0 + 128], ident)
                        nc.vector.tensor_copy(v_sb[:, st * 4 + j, :], vp)

            # ---------------- Phase B: attention ----------------
            with tc.tile_pool(name="e", bufs=3) as e_pool, \
                 tc.tile_pool(name="usb", bufs=2) as usb, \
                 tc.tile_pool(name="rsb", bufs=2) as rsb, \
                 tc.tile_pool(name="osb", bufs=4) as osb, \
                 tc.tile_pool(name="sps", bufs=2, space="PSUM") as sps, \
                 tc.tile_pool(name="ops", bufs=1, space="PSUM") as ops, \
                 tc.tile_pool(name="rps", bufs=1, space="PSUM") as rps:
                for qt in range(NQT):
                    q0 = qt * QT
                    o1_ps = ops.tile([128, QT], F32, tag="o1")
                    o2_ps = ops.tile([128, QT], F32, tag="o2")
                    r1_ps = rps.tile([1, QT], F32, tag="r1")
                    r2_ps = rps.tile([1, QT], F32, tag="r2")
                    for kt in range(NKT):
                        k0 = kt * KT
                        s12 = sps.tile([128, 2, QT], F32, tag="s")
                        nc.tensor.matmul(
                            s12[:, 0, :], kT[0:64, k0:k0 + KT],
                            qT[0:64, q0:q0 + QT], start=True, stop=True)
                        nc.tensor.matmul(
                            s12[:, 1, :], kT[64:128, k0:k0 + KT],
                            qT[64:128, q0:q0 + QT], start=True, stop=True)
                        e12 = e_pool.tile([128, 2, QT], F32R, tag="e")
                        nc.scalar.activation(
                            out=e12, in_=s12, func=AF.Exp, scale=0.125)
                        first, last = (kt == 0), (kt == NKT - 1)
                        nc.tensor.matmul(o1_ps, v_sb[:, kt, :], e12[:, 0, :],
                                         start=first, stop=last)
                        nc.tensor.matmul(o2_ps, v_sb[:, kt, :], e12[:, 1, :],
                                         start=first, stop=last)
                        nc.tensor.matmul(r1_ps, ones, e12[:, 0, :],
                                         start=first, stop=last)
                        nc.tensor.matmul(r2_ps, ones, e12[:, 1, :],
                                         start=first, stop=last)

                    # epilogue: normalize and write out
                    u1 = usb.tile([128, QT], F32, tag="u1")
                    u2 = usb.tile([128, QT], F32, tag="u2")
                    nc.vector.tensor_copy(u1, o1_ps)
                    nc.vector.tensor_copy(u2, o2_ps)
                    ri1 = rsb.tile([1, QT], F32, tag="ri1")
                    ri2 = rsb.tile([1, QT], F32, tag="ri2")
                    nc.vector.reciprocal(ri1, r1_ps)
                    nc.vector.reciprocal(ri2, r2_ps)
                    for j in range(QT // 128):
                        c0 = j * 128
                        rt1 = rps.tile([128, 1], F32, tag="r1")
                        rt2 = rps.tile([128, 1], F32, tag="r2")
                        nc.tensor.transpose(rt1, ri1[0:1, c0:c0 + 128], ident1)
                        nc.tensor.transpose(rt2, ri2[0:1, c0:c0 + 128], ident1)
                        rc1 = osb.tile([128, 1], F32, tag="rc1")
                        rc2 = osb.tile([128, 1], F32, tag="rc2")
                        nc.vector.tensor_copy(rc1, rt1)
                        # rc2 = lam / r2
                        nc.vector.tensor_mul(rc2, rt2, lam_sb)
                        ut1 = ops.tile([128, TD], F32, tag="o1")
                        ut2 = ops.tile([128, TD], F32, tag="o2")
                        nc.tensor.transpose(ut1, u1[:, c0:c0 + 128], ident)
                        nc.tensor.transpose(ut2, u2[:, c0:c0 + 128], ident)
                        tmp = osb.tile([128, TD], F32, tag="tmp")
                        nc.vector.tensor_scalar_mul(tmp, ut2, rc2)
                        ob = osb.tile([128, TD], F32, tag="ob")
                        nc.vector.scalar_tensor_tensor(
                            out=ob, in0=ut1, scalar=rc1, in1=tmp,
                            op0=mybir.AluOpType.mult,
                            op1=mybir.AluOpType.subtract)
                        nc.sync.dma_start(
                            out=out_t.ap()[q0 + c0:q0 + c0 + 128, :], in_=ob)

    nc.compile()
    return nc


_NC_CACHE = None


def kernel(X, W_q, W_k, W_v, lam):
    global _NC_CACHE
    from concourse.bass_utils import run_bass_kernel_spmd

    X = np.asarray(X, dtype=np.float32)
    W_q = np.asarray(W_q, dtype=np.float32)
    W_k = np.asarray(W_k, dtype=np.float32)
    W_v = np.asarray(W_v, dtype=np.float32)
    lam_arr = np.asarray(lam, dtype=np.float32).reshape(1, 1)

    if _NC_CACHE is None:
        _NC_CACHE = build_nc()
    nc = _NC_CACHE

    in_maps = []
    for c in range(8):
        b, qc = divmod(c, 4)
        qs = qc * NQ
        Xc = np.ascontiguousarray(
            np.concatenate([X[b, qs:], X[b, :qs]], axis=0))
        in_maps.append({"X": Xc, "Wq": W_q, "Wk": W_k, "Wv": W_v,
                        "lam": lam_arr})

    res = run_bass_kernel_spmd(nc, in_maps, core_ids=list(range(8)))

    out = np.empty((B, S, TD), dtype=np.float32)
    for c in range(8):
        b, qc = divmod(c, 4)
        qs = qc * NQ
        out[b, qs:qs + NQ] = res.results[c]["out"]
    return out
